# revision 31
# baseline (speedup 1.0000x reference)
"""FWHT (normalized Walsh-Hadamard transform) over the last dim of x[4,4096,4096].

Rows are independent -> shard 16384 rows across 8 NeuronCores (2048 each).
Per row, H_4096 = H_32 (x) H_128 (Sylvester Kronecker factorization); with
the row viewed as X[c, k] (c in [0,32), k in [0,128)):

    y[c'*128+k'] = (1/64) * sum_{c,k} H32[c',c] H128[k',k] X[c,k]

All storage and PE streams are bf16 (inputs cast on host; rel err ~2e-3,
inside the 2e-2 gate); PSUM accumulation is fp32.  Work is organized in
16 half-blocks of 128 rows, software-pipelined so that every engine
(PE / DVE / ACT / DMA) stays busy:

  T0 :  PE transposes x[r, (c,k)] tiles  -> Z[k | (g,c,rr)]   bf16  (DVE drain,
        scattered so stage A output is g-contiguous; r = g*4+rr)
  A  :  matmul H128/8 x Z                -> B[k' | (g,c,rr)]  (ACT drain, cast)
  T1 :  PE transposes B[:, g*128:+128]   -> Z2[(c,rr) | k']   bf16  (DVE drain)
  B  :  matmul kron(H32,I4)/8 x Z2       -> OUT[(c',rr') | (g,k')] f32
  out:  4 DMAs per half (one per rr), 512B-contiguous k' runs.

Emission interleaves, per half-block: [A,A,T1+B] x4 with the NEXT half's
T0 tiles injected each iteration, so DVE (bf16 drains) and ACT (fp32
drains) overlap instead of alternating in phases.
"""

import numpy as np

try:
    import concourse.bass as bass  # noqa: F401
except ImportError:
    import sys

    sys.path.insert(0, "/opt/trn_rl_repo")

from concourse import bacc, bass, bass_utils, tile
from concourse import mybir

F32 = mybir.dt.float32
BF16 = mybir.dt.bfloat16

N_CORES = 8
ROWS_PER_CORE = 2048
DIM = 4096
N_BLOCKS = 8
BLOCK_ROWS = 256
N_HALVES = 16  # 128-row halves per pass


def _hadamard(n: int) -> np.ndarray:
    h = np.array([[1.0]], dtype=np.float64)
    while h.shape[0] < n:
        h = np.block([[h, h], [h, -h]])
    return h


def _constants():
    import ml_dtypes

    bf = ml_dtypes.bfloat16
    h128 = (_hadamard(128) * 0.125).astype(bf)
    # stage-B stationary: contraction index p=(c*4+rr), output q=(c'*4+rr')
    k32 = (np.kron(_hadamard(32), np.eye(4)) * 0.125).astype(bf)
    ident = np.eye(128).astype(bf)
    return h128, k32, ident


def build_program(repeat: int = 1, hw_loop: bool = False):
    nc = bacc.Bacc(
        "TRN2",
        target_bir_lowering=False,
        debug=False,
        enable_asserts=False,
    )

    x_d = nc.dram_tensor("x", [ROWS_PER_CORE, DIM], BF16, kind="ExternalInput").ap()
    h128_d = nc.dram_tensor("h128", [128, 128], BF16, kind="ExternalInput").ap()
    k32_d = nc.dram_tensor("k32", [128, 128], BF16, kind="ExternalInput").ap()
    id_d = nc.dram_tensor("ident", [128, 128], BF16, kind="ExternalInput").ap()
    # y is written bf16 in the SBUF-native layout [(c'*4+rr'), half, g*128+k']
    # (fully contiguous per partition -> 8KB DMA descriptors); the host
    # permutes/upcasts during unshard.
    y_d = nc.dram_tensor(
        "y", [128, N_HALVES, DIM], BF16, kind="ExternalOutput"
    ).ap()

    with tile.TileContext(nc) as tc:
        with (
            tc.tile_pool(name="const", bufs=1) as const_pool,
            tc.tile_pool(name="xin", bufs=6) as x_pool,
            tc.tile_pool(name="zbuf", bufs=3) as z_pool,
            tc.tile_pool(name="bbuf", bufs=2) as b_pool,
            tc.tile_pool(name="z2buf", bufs=3) as z2_pool,
            tc.tile_pool(name="obuf", bufs=3) as o_pool,
            tc.tile_pool(name="ps_t0", bufs=2, space="PSUM") as t0_psum,
            tc.tile_pool(name="ps_a", bufs=1, space="PSUM") as a_psum,
            tc.tile_pool(name="ps_t1", bufs=2, space="PSUM") as t1_psum,
            tc.tile_pool(name="ps_b", bufs=1, space="PSUM") as b_psum,
        ):
            h128_t = const_pool.tile([128, 128], BF16)
            k32_t = const_pool.tile([128, 128], BF16)
            id_t = const_pool.tile([128, 128], BF16)
            nc.sync.dma_start(h128_t[:], h128_d)
            nc.sync.dma_start(k32_t[:], k32_d)
            nc.sync.dma_start(id_t[:], id_d)

            xts = {}  # half -> x tile [128, 4096]
            z_tiles = {}  # half -> Z tile
            bb_tiles = {}
            out_tiles = {}

            def emit_in(h):
                b, rs = divmod(h, 2)
                t = x_pool.tile([128, DIM], BF16, name=f"x_{h}", tag="x")
                eng = nc.sync if rs == 0 else nc.scalar
                r0 = (b % N_BLOCKS) * BLOCK_ROWS + rs * 128
                eng.dma_start(t[:], x_d[r0 : r0 + 128, :])
                xts[h] = t

            def emit_t0(h, j):
                # j-th T0 psum tile = 8 transposes, c in [8j, 8j+8)
                if j == 0:
                    z_tiles[h] = z_pool.tile([128, DIM], BF16, name=f"z_{h}", tag="z")
                z = z_tiles[h]
                xt = xts.pop(h) if j == 3 else xts[h]
                ps = t0_psum.tile([128, 1024], BF16, name=f"t0ps_{h}_{j}", tag="t0ps")
                for i in range(8):
                    c = j * 8 + i
                    nc.tensor.transpose(
                        ps[:, i * 128 : (i + 1) * 128],
                        xt[:, c * 128 : (c + 1) * 128],
                        id_t[:],
                    )
                zr = z[:].rearrange("p (g c rr) -> p c g rr", g=32, c=32, rr=4)
                psr = ps[:].rearrange("p (c g rr) -> p c g rr", c=8, g=32, rr=4)
                nc.vector.tensor_copy(zr[:, j * 8 : (j + 1) * 8], psr)

            def emit_a(h, mp):
                # mp-th A pair: 2 matmuls into one 2-bank PSUM tile, 1 drain
                if mp == 0:
                    bb_tiles[h] = b_pool.tile([128, DIM], BF16, name=f"bb_{h}", tag="bb")
                z = z_tiles[h]
                bb = bb_tiles[h]
                ps = a_psum.tile([128, 1024], F32, name=f"aps_{h}_{mp}", tag="aps")
                for q in range(2):
                    m = mp * 2 + q
                    nc.tensor.matmul(
                        ps[:, q * 512 : (q + 1) * 512],
                        h128_t[:],
                        z[:, m * 512 : (m + 1) * 512],
                    )
                nc.scalar.copy(bb[:, mp * 1024 : (mp + 1) * 1024], ps[:])
                if mp == 3:
                    del z_tiles[h]

            z2_tiles = {}

            def emit_t1(h, t):
                bb = bb_tiles[h]
                ps = t1_psum.tile([128, 1024], BF16, name=f"t1ps_{h}_{t}", tag="t1ps")
                for j in range(8):
                    g = t * 8 + j
                    nc.tensor.transpose(
                        ps[:, j * 128 : (j + 1) * 128],
                        bb[:, g * 128 : (g + 1) * 128],
                        id_t[:],
                    )
                z2 = z2_pool.tile([128, 1024], BF16, name=f"z2_{h}_{t}", tag="z2")
                nc.vector.tensor_copy(z2[:], ps[:])
                z2_tiles[(h, t)] = z2
                if t == 3:
                    del bb_tiles[h]

            def emit_b(h, t):
                if t == 0:
                    out_tiles[h] = o_pool.tile([128, DIM], BF16, name=f"out_{h}", tag="out")
                out = out_tiles[h]
                z2 = z2_tiles.pop((h, t))
                psb = b_psum.tile([128, 1024], F32, name=f"bps_{h}_{t}", tag="bps")
                for hh in range(2):
                    nc.tensor.matmul(
                        psb[:, hh * 512 : (hh + 1) * 512],
                        k32_t[:],
                        z2[:, hh * 512 : (hh + 1) * 512],
                    )
                dst = out[:, t * 1024 : (t + 1) * 1024]
                # ~1/3 of the fp32 B-drains go to DVE, the rest to ACT
                if (h * 4 + t) % 3 == 0:
                    nc.vector.tensor_copy(dst, psb[:])
                else:
                    nc.scalar.copy(dst, psb[:])
                if t == 3:
                    emit_out(h)

            def emit_out(h):
                b, rs = divmod(h, 2)
                out = out_tiles.pop(h)
                eng = nc.sync if h % 2 == 0 else nc.scalar
                eng.dma_start(y_d[:, h % N_HALVES, :], out[:])

            def emit_pass():
                # 64 pipeline slots; unit s = (H=s//4, t=s%4).  Leads/lags:
                # T0 of half h is injected 2 slots ahead of h's first A-pair;
                # A-pair for unit u is emitted 2 slots before its T1;
                # B runs 1 slot behind its T1.
                units = [(H, t) for H in range(N_HALVES) for t in range(4)]

                emit_in(0)
                emit_in(1)
                emit_in(2)
                for j in range(4):
                    emit_t0(0, j)
                emit_a(0, 0)
                emit_a(0, 1)
                emit_a(0, 2)
                emit_a(0, 3)
                emit_t0(1, 0)
                emit_t0(1, 1)
                for s in range(len(units)):
                    H, t = units[s]
                    if t == 0 and H >= 1 and H + 2 < N_HALVES:
                        emit_in(H + 2)
                    emit_t1(H, t)
                    # inject T0 tile (h', j') solving (h'-1)*4 + j' - 2 == s
                    j2 = (s + 2) % 4
                    h2 = (s + 2 - j2) // 4 + 1
                    if h2 < N_HALVES:
                        emit_t0(h2, j2)
                    if s >= 1:
                        emit_b(*units[s - 1])
                    if s + 2 < len(units):
                        H3, t3 = units[s + 2]
                        emit_a(H3, 2 * t3)
                        emit_a(H3, 2 * t3 + 1)
                emit_b(*units[-1])

            import contextlib

            loop_ctx = (
                tc.For_i(0, repeat) if hw_loop and repeat > 1
                else contextlib.nullcontext()
            )
            with loop_ctx:
                for _ in range(1 if hw_loop else repeat):
                    emit_pass()

    nc.compile()
    return nc


_CACHE = {}


def _get_program():
    if "nc" not in _CACHE:
        _CACHE["nc"] = build_program()
    return _CACHE["nc"]


def make_in_maps(x: np.ndarray) -> list:
    import ml_dtypes

    x_flat = np.ascontiguousarray(
        x.reshape(16384, DIM).astype(ml_dtypes.bfloat16)
    )
    h128, k32, ident = _constants()
    return [
        {
            "x": x_flat[i * ROWS_PER_CORE : (i + 1) * ROWS_PER_CORE],
            "h128": h128,
            "k32": k32,
            "ident": ident,
        }
        for i in range(N_CORES)
    ]


def assemble_output(res: dict) -> np.ndarray:
    """Undo the packed per-core output layout: y_packed[core] is
    [(c'*4+rr'), half, g*128+k'] bf16; global row = half*128 + g*4 + rr'."""
    yp = np.asarray(res["y"]).reshape(N_CORES, 32, 4, N_HALVES, 32, 128)
    y = yp.transpose(0, 3, 4, 2, 1, 5).reshape(16384, 4096)
    return y.astype(np.float32).reshape(4, 4096, 4096)


def kernel(x: np.ndarray, _trace: bool = False, _trace_kwargs=None) -> np.ndarray:
    assert x.shape == (4, 4096, 4096), x.shape
    in_maps = make_in_maps(x)

    nc = _get_program()
    res = bass_utils.run_bass_kernel_spmd(
        nc,
        in_maps,
        core_ids=list(range(N_CORES)),
        trace=_trace,
        **(_trace_kwargs or {}),
    )
    y = assemble_output({"y": np.stack([res.results[i]["y"] for i in range(N_CORES)])})
    if _trace:
        _CACHE["last_result"] = res
    return y


# revision 36
# speedup vs baseline: 1.3402x; 1.3402x over previous
"""FWHT (normalized Walsh-Hadamard transform) over the last dim of x[4,4096,4096].

Rows are independent -> shard 16384 rows across 8 NeuronCores (2048 each).
Per row, H_4096 = H_32 (x) H_128 (Sylvester Kronecker factorization); with
the row viewed as X[c, k] (c in [0,32), k in [0,128)):

    y[c'*128+k'] = (1/64) * sum_{c,k} H32[c',c] H128[k',k] X[c,k]

All storage and PE streams are bf16 (inputs cast on host; rel err ~2e-3,
inside the 2e-2 gate); PSUM accumulation is fp32.  Work is organized in
16 half-blocks of 128 rows, software-pipelined so that every engine
(PE / DVE / ACT / DMA) stays busy:

  T0 :  PE transposes x[r, (c,k)] tiles  -> Z[k | (g,c,rr)]   bf16  (DVE drain,
        scattered so stage A output is g-contiguous; r = g*4+rr)
  A  :  matmul H128/8 x Z                -> B[k' | (g,c,rr)]  (ACT drain, cast)
  T1 :  PE transposes B[:, g*128:+128]   -> Z2[(c,rr) | k']   bf16  (DVE drain)
  B  :  matmul kron(H32,I4)/8 x Z2       -> OUT[(c',rr') | (g,k')] f32
  out:  4 DMAs per half (one per rr), 512B-contiguous k' runs.

Emission interleaves, per half-block: [A,A,T1+B] x4 with the NEXT half's
T0 tiles injected each iteration, so DVE (bf16 drains) and ACT (fp32
drains) overlap instead of alternating in phases.
"""

import numpy as np

try:
    import concourse.bass as bass  # noqa: F401
except ImportError:
    import sys

    sys.path.insert(0, "/opt/trn_rl_repo")

from concourse import bacc, bass, bass_utils, tile
from concourse import mybir

F32 = mybir.dt.float32
BF16 = mybir.dt.bfloat16

N_CORES = 8
ROWS_PER_CORE = 2048
DIM = 4096
N_BLOCKS = 8
BLOCK_ROWS = 256
N_HALVES = 16  # 128-row halves per pass


def _hadamard(n: int) -> np.ndarray:
    h = np.array([[1.0]], dtype=np.float64)
    while h.shape[0] < n:
        h = np.block([[h, h], [h, -h]])
    return h


def _constants():
    import ml_dtypes

    bf = ml_dtypes.bfloat16
    h128 = (_hadamard(128) * 0.125).astype(bf)
    # stage-B stationary: contraction index p=(c*4+rr), output q=(c'*4+rr')
    k32 = (np.kron(_hadamard(32), np.eye(4)) * 0.125).astype(bf)
    ident = np.eye(128).astype(bf)
    return h128, k32, ident


def build_program(repeat: int = 1, hw_loop: bool = False):
    nc = bacc.Bacc(
        "TRN2",
        target_bir_lowering=False,
        debug=False,
        enable_asserts=False,
    )

    x_d = nc.dram_tensor("x", [ROWS_PER_CORE, DIM], BF16, kind="ExternalInput").ap()
    h128_d = nc.dram_tensor("h128", [128, 128], BF16, kind="ExternalInput").ap()
    k32_d = nc.dram_tensor("k32", [128, 128], BF16, kind="ExternalInput").ap()
    id_d = nc.dram_tensor("ident", [128, 128], BF16, kind="ExternalInput").ap()
    # y is written bf16 in the SBUF-native layout [(c'*4+rr'), half, g*128+k']
    # (fully contiguous per partition -> 8KB DMA descriptors); the host
    # permutes/upcasts during unshard.
    y_d = nc.dram_tensor(
        "y", [128, N_HALVES, DIM], BF16, kind="ExternalOutput"
    ).ap()

    with tile.TileContext(nc) as tc:
        with (
            tc.tile_pool(name="const", bufs=1) as const_pool,
            tc.tile_pool(name="xin", bufs=6) as x_pool,
            tc.tile_pool(name="zbuf", bufs=3) as z_pool,
            tc.tile_pool(name="bbuf", bufs=2) as b_pool,
            tc.tile_pool(name="z2buf", bufs=3) as z2_pool,
            tc.tile_pool(name="obuf", bufs=3) as o_pool,
            tc.tile_pool(name="ps_t0", bufs=2, space="PSUM") as t0_psum,
            tc.tile_pool(name="ps_a", bufs=2, space="PSUM") as a_psum,
            tc.tile_pool(name="ps_t1", bufs=2, space="PSUM") as t1_psum,
            tc.tile_pool(name="ps_b", bufs=2, space="PSUM") as b_psum,
        ):
            h128_t = const_pool.tile([128, 128], BF16)
            k32_t = const_pool.tile([128, 128], BF16)
            id_t = const_pool.tile([128, 128], BF16)
            nc.sync.dma_start(h128_t[:], h128_d)
            nc.sync.dma_start(k32_t[:], k32_d)
            nc.sync.dma_start(id_t[:], id_d)

            xts = {}  # half -> x tile [128, 4096]
            z_tiles = {}  # half -> Z tile
            bb_tiles = {}
            out_tiles = {}

            def emit_in(h):
                b, rs = divmod(h, 2)
                t = x_pool.tile([128, DIM], BF16, name=f"x_{h}", tag="x")
                eng = nc.sync if rs == 0 else nc.scalar
                r0 = (b % N_BLOCKS) * BLOCK_ROWS + rs * 128
                eng.dma_start(t[:], x_d[r0 : r0 + 128, :])
                xts[h] = t

            def emit_t0(h, j):
                # j-th T0 psum tile = 8 transposes, c in [8j, 8j+8)
                if j == 0:
                    z_tiles[h] = z_pool.tile([128, DIM], BF16, name=f"z_{h}", tag="z")
                z = z_tiles[h]
                xt = xts.pop(h) if j == 3 else xts[h]
                ps = t0_psum.tile([128, 1024], BF16, name=f"t0ps_{h}_{j}", tag="t0ps")
                for i in range(8):
                    c = j * 8 + i
                    nc.tensor.transpose(
                        ps[:, i * 128 : (i + 1) * 128],
                        xt[:, c * 128 : (c + 1) * 128],
                        id_t[:],
                    )
                zr = z[:].rearrange("p (g c rr) -> p c g rr", g=32, c=32, rr=4)
                psr = ps[:].rearrange("p (c g rr) -> p c g rr", c=8, g=32, rr=4)
                nc.vector.tensor_copy(zr[:, j * 8 : (j + 1) * 8], psr)

            def emit_a(h, m):
                if m == 0:
                    bb_tiles[h] = b_pool.tile([128, DIM], BF16, name=f"bb_{h}", tag="bb")
                z = z_tiles[h]
                bb = bb_tiles[h]
                ps = a_psum.tile([128, 512], F32, name=f"aps_{h}_{m}", tag="aps")
                nc.tensor.matmul(ps[:], h128_t[:], z[:, m * 512 : (m + 1) * 512])
                nc.scalar.copy(bb[:, m * 512 : (m + 1) * 512], ps[:])
                if m == 7:
                    del z_tiles[h]

            z2_tiles = {}

            def emit_t1(h, t):
                bb = bb_tiles[h]
                ps = t1_psum.tile([128, 1024], BF16, name=f"t1ps_{h}_{t}", tag="t1ps")
                for j in range(8):
                    g = t * 8 + j
                    nc.tensor.transpose(
                        ps[:, j * 128 : (j + 1) * 128],
                        bb[:, g * 128 : (g + 1) * 128],
                        id_t[:],
                    )
                z2 = z2_pool.tile([128, 1024], BF16, name=f"z2_{h}_{t}", tag="z2")
                nc.vector.tensor_copy(z2[:], ps[:])
                z2_tiles[(h, t)] = z2
                if t == 3:
                    del bb_tiles[h]

            def emit_b(h, t):
                if t == 0:
                    out_tiles[h] = o_pool.tile([128, DIM], BF16, name=f"out_{h}", tag="out")
                out = out_tiles[h]
                z2 = z2_tiles.pop((h, t))
                for hh in range(2):
                    psb = b_psum.tile([128, 512], F32, name=f"bps_{h}_{t}_{hh}", tag="bps")
                    nc.tensor.matmul(
                        psb[:], k32_t[:], z2[:, hh * 512 : (hh + 1) * 512]
                    )
                    dst = out[:, t * 1024 + hh * 512 : t * 1024 + (hh + 1) * 512]
                    # 4 of 8 fp32 B-drains go to DVE, the rest to ACT
                    if (t * 2 + hh) % 2 == 0:
                        nc.vector.tensor_copy(dst, psb[:])
                    else:
                        nc.scalar.copy(dst, psb[:])
                if t == 3:
                    emit_out(h)

            def emit_out(h):
                b, rs = divmod(h, 2)
                out = out_tiles.pop(h)
                eng = nc.sync if h % 2 == 0 else nc.scalar
                eng.dma_start(y_d[:, h % N_HALVES, :], out[:])

            def emit_pass():
                # 64 pipeline slots; unit s = (H=s//4, t=s%4).  Leads/lags:
                # T0 of half h is injected 2 slots ahead of h's first A-pair;
                # A-pair for unit u is emitted 2 slots before its T1;
                # B runs 1 slot behind its T1.
                units = [(H, t) for H in range(N_HALVES) for t in range(4)]

                emit_in(0)
                emit_in(1)
                emit_in(2)
                for j in range(4):
                    emit_t0(0, j)
                emit_a(0, 0)
                emit_a(0, 1)
                emit_a(0, 2)
                emit_a(0, 3)
                emit_t0(1, 0)
                emit_t0(1, 1)
                for s in range(len(units)):
                    H, t = units[s]
                    if t == 0 and H >= 1 and H + 2 < N_HALVES:
                        emit_in(H + 2)
                    emit_t1(H, t)
                    # inject T0 tile (h', j') solving (h'-1)*4 + j' - 2 == s
                    j2 = (s + 2) % 4
                    h2 = (s + 2 - j2) // 4 + 1
                    if h2 < N_HALVES:
                        emit_t0(h2, j2)
                    if s >= 1:
                        emit_b(*units[s - 1])
                    if s + 2 < len(units):
                        H3, t3 = units[s + 2]
                        emit_a(H3, 2 * t3)
                        emit_a(H3, 2 * t3 + 1)
                emit_b(*units[-1])

            import contextlib

            loop_ctx = (
                tc.For_i(0, repeat) if hw_loop and repeat > 1
                else contextlib.nullcontext()
            )
            with loop_ctx:
                for _ in range(1 if hw_loop else repeat):
                    emit_pass()

    nc.compile()
    return nc


_CACHE = {}


def _get_program():
    if "nc" not in _CACHE:
        _CACHE["nc"] = build_program()
    return _CACHE["nc"]


def make_in_maps(x: np.ndarray) -> list:
    import ml_dtypes

    x_flat = np.ascontiguousarray(
        x.reshape(16384, DIM).astype(ml_dtypes.bfloat16)
    )
    h128, k32, ident = _constants()
    return [
        {
            "x": x_flat[i * ROWS_PER_CORE : (i + 1) * ROWS_PER_CORE],
            "h128": h128,
            "k32": k32,
            "ident": ident,
        }
        for i in range(N_CORES)
    ]


def assemble_output(res: dict) -> np.ndarray:
    """Undo the packed per-core output layout: y_packed[core] is
    [(c'*4+rr'), half, g*128+k'] bf16; global row = half*128 + g*4 + rr'."""
    yp = np.asarray(res["y"]).reshape(N_CORES, 32, 4, N_HALVES, 32, 128)
    y = yp.transpose(0, 3, 4, 2, 1, 5).reshape(16384, 4096)
    return y.astype(np.float32).reshape(4, 4096, 4096)


def kernel(x: np.ndarray, _trace: bool = False, _trace_kwargs=None) -> np.ndarray:
    assert x.shape == (4, 4096, 4096), x.shape
    in_maps = make_in_maps(x)

    nc = _get_program()
    res = bass_utils.run_bass_kernel_spmd(
        nc,
        in_maps,
        core_ids=list(range(N_CORES)),
        trace=_trace,
        **(_trace_kwargs or {}),
    )
    y = assemble_output({"y": np.stack([res.results[i]["y"] for i in range(N_CORES)])})
    if _trace:
        _CACHE["last_result"] = res
    return y


# revision 40
# speedup vs baseline: 1.3990x; 1.0438x over previous
"""FWHT (normalized Walsh-Hadamard transform) over the last dim of x[4,4096,4096].

Rows are independent -> shard 16384 rows across 8 NeuronCores (2048 each).
Per row, H_4096 = H_32 (x) H_128 (Sylvester Kronecker factorization); with
the row viewed as X[c, k] (c in [0,32), k in [0,128)):

    y[c'*128+k'] = (1/64) * sum_{c,k} H32[c',c] H128[k',k] X[c,k]

All storage and PE streams are bf16 (inputs cast on host; rel err ~2e-3,
inside the 2e-2 gate); PSUM accumulation is fp32.  Work is organized in
16 half-blocks of 128 rows, software-pipelined so that every engine
(PE / DVE / ACT / DMA) stays busy:

  T0 :  PE transposes x[r, (c,k)] tiles  -> Z[k | (g,c,rr)]   bf16  (DVE drain,
        scattered so stage A output is g-contiguous; r = g*4+rr)
  A  :  matmul H128/8 x Z                -> B[k' | (g,c,rr)]  (ACT drain, cast)
  T1 :  PE transposes B[:, g*128:+128]   -> Z2[(c,rr) | k']   bf16  (DVE drain)
  B  :  matmul kron(H32,I4)/8 x Z2       -> OUT[(c',rr') | (g,k')] f32
  out:  4 DMAs per half (one per rr), 512B-contiguous k' runs.

Emission interleaves, per half-block: [A,A,T1+B] x4 with the NEXT half's
T0 tiles injected each iteration, so DVE (bf16 drains) and ACT (fp32
drains) overlap instead of alternating in phases.
"""

import numpy as np

try:
    import concourse.bass as bass  # noqa: F401
except ImportError:
    import sys

    sys.path.insert(0, "/opt/trn_rl_repo")

from concourse import bacc, bass, bass_utils, tile
from concourse import mybir

F32 = mybir.dt.float32
BF16 = mybir.dt.bfloat16

N_CORES = 8
ROWS_PER_CORE = 2048
DIM = 4096
N_BLOCKS = 8
BLOCK_ROWS = 256
N_HALVES = 16  # 128-row halves per pass


def _hadamard(n: int) -> np.ndarray:
    h = np.array([[1.0]], dtype=np.float64)
    while h.shape[0] < n:
        h = np.block([[h, h], [h, -h]])
    return h


def _constants():
    import ml_dtypes

    bf = ml_dtypes.bfloat16
    h128 = (_hadamard(128) * 0.125).astype(bf)
    # stage-B stationary: contraction index p=(c*4+rr), output q=(c'*4+rr')
    k32 = (np.kron(_hadamard(32), np.eye(4)) * 0.125).astype(bf)
    ident = np.eye(128).astype(bf)
    return h128, k32, ident


def build_program(repeat: int = 1, hw_loop: bool = False):
    nc = bacc.Bacc(
        "TRN2",
        target_bir_lowering=False,
        debug=False,
        enable_asserts=False,
    )

    x_d = nc.dram_tensor("x", [ROWS_PER_CORE, DIM], BF16, kind="ExternalInput").ap()
    h128_d = nc.dram_tensor("h128", [128, 128], BF16, kind="ExternalInput").ap()
    k32_d = nc.dram_tensor("k32", [128, 128], BF16, kind="ExternalInput").ap()
    id_d = nc.dram_tensor("ident", [128, 128], BF16, kind="ExternalInput").ap()
    # y is written bf16 in the SBUF-native layout [(c'*4+rr'), half, g*128+k']
    # (fully contiguous per partition -> 8KB DMA descriptors); the host
    # permutes/upcasts during unshard.
    y_d = nc.dram_tensor(
        "y", [128, N_HALVES, DIM], BF16, kind="ExternalOutput"
    ).ap()

    with tile.TileContext(nc) as tc:
        with (
            tc.tile_pool(name="const", bufs=1) as const_pool,
            tc.tile_pool(name="xin", bufs=6) as x_pool,
            tc.tile_pool(name="zbuf", bufs=3) as z_pool,
            tc.tile_pool(name="bbuf", bufs=2) as b_pool,
            tc.tile_pool(name="z2buf", bufs=3) as z2_pool,
            tc.tile_pool(name="obuf", bufs=3) as o_pool,
            tc.tile_pool(name="ps_t", bufs=3, space="PSUM") as t_psum,
            tc.tile_pool(name="ps_a", bufs=3, space="PSUM") as a_psum,
            tc.tile_pool(name="ps_b", bufs=2, space="PSUM") as b_psum,
        ):
            h128_t = const_pool.tile([128, 128], BF16)
            k32_t = const_pool.tile([128, 128], BF16)
            id_t = const_pool.tile([128, 128], BF16)
            nc.sync.dma_start(h128_t[:], h128_d)
            nc.sync.dma_start(k32_t[:], k32_d)
            nc.sync.dma_start(id_t[:], id_d)

            xts = {}  # half -> x tile [128, 4096]
            z_tiles = {}  # half -> Z tile
            bb_tiles = {}
            out_tiles = {}

            def emit_in(h):
                b, rs = divmod(h, 2)
                t = x_pool.tile([128, DIM], BF16, name=f"x_{h}", tag="x")
                eng = nc.sync if rs == 0 else nc.scalar
                r0 = (b % N_BLOCKS) * BLOCK_ROWS + rs * 128
                eng.dma_start(t[:], x_d[r0 : r0 + 128, :])
                xts[h] = t

            def emit_t0(h, j):
                # j-th T0 psum tile = 8 transposes, c in [8j, 8j+8)
                if j == 0:
                    z_tiles[h] = z_pool.tile([128, DIM], BF16, name=f"z_{h}", tag="z")
                z = z_tiles[h]
                xt = xts.pop(h) if j == 3 else xts[h]
                ps = t_psum.tile([128, 1024], BF16, name=f"t0ps_{h}_{j}", tag="tps")
                for i in range(8):
                    c = j * 8 + i
                    nc.tensor.transpose(
                        ps[:, i * 128 : (i + 1) * 128],
                        xt[:, c * 128 : (c + 1) * 128],
                        id_t[:],
                    )
                zr = z[:].rearrange("p (g c rr) -> p c g rr", g=32, c=32, rr=4)
                psr = ps[:].rearrange("p (c g rr) -> p c g rr", c=8, g=32, rr=4)
                nc.vector.tensor_copy(zr[:, j * 8 : (j + 1) * 8], psr)

            def emit_a(h, m):
                if m == 0:
                    bb_tiles[h] = b_pool.tile([128, DIM], BF16, name=f"bb_{h}", tag="bb")
                z = z_tiles[h]
                bb = bb_tiles[h]
                ps = a_psum.tile([128, 512], F32, name=f"aps_{h}_{m}", tag="aps")
                nc.tensor.matmul(ps[:], h128_t[:], z[:, m * 512 : (m + 1) * 512])
                nc.scalar.copy(bb[:, m * 512 : (m + 1) * 512], ps[:])
                if m == 7:
                    del z_tiles[h]

            z2_tiles = {}

            def emit_t1(h, t):
                bb = bb_tiles[h]
                ps = t_psum.tile([128, 1024], BF16, name=f"t1ps_{h}_{t}", tag="tps")
                for j in range(8):
                    g = t * 8 + j
                    nc.tensor.transpose(
                        ps[:, j * 128 : (j + 1) * 128],
                        bb[:, g * 128 : (g + 1) * 128],
                        id_t[:],
                    )
                z2 = z2_pool.tile([128, 1024], BF16, name=f"z2_{h}_{t}", tag="z2")
                nc.vector.tensor_copy(z2[:], ps[:])
                z2_tiles[(h, t)] = z2
                if t == 3:
                    del bb_tiles[h]

            def emit_b(h, t):
                if t == 0:
                    out_tiles[h] = o_pool.tile([128, DIM], BF16, name=f"out_{h}", tag="out")
                out = out_tiles[h]
                z2 = z2_tiles.pop((h, t))
                for hh in range(2):
                    psb = b_psum.tile([128, 512], F32, name=f"bps_{h}_{t}_{hh}", tag="bps")
                    nc.tensor.matmul(
                        psb[:], k32_t[:], z2[:, hh * 512 : (hh + 1) * 512]
                    )
                    dst = out[:, t * 1024 + hh * 512 : t * 1024 + (hh + 1) * 512]
                    # ~4/9 of fp32 B-drains to DVE balances DVE/ACT busy time
                    if ((h * 4 + t) * 2 + hh) % 9 < 4:
                        nc.vector.tensor_copy(dst, psb[:])
                    else:
                        nc.scalar.copy(dst, psb[:])
                if t == 3:
                    emit_out(h)

            def emit_out(h):
                b, rs = divmod(h, 2)
                out = out_tiles.pop(h)
                eng = nc.sync if h % 2 == 0 else nc.scalar
                eng.dma_start(y_d[:, h % N_HALVES, :], out[:])

            def emit_pass():
                # 64 pipeline slots; unit s = (H=s//4, t=s%4).  Leads/lags:
                # T0 of half h is injected 2 slots ahead of h's first A-pair;
                # A-pair for unit u is emitted 2 slots before its T1;
                # B runs 1 slot behind its T1.
                units = [(H, t) for H in range(N_HALVES) for t in range(4)]

                emit_in(0)
                emit_in(1)
                emit_in(2)
                for j in range(4):
                    emit_t0(0, j)
                emit_a(0, 0)
                emit_a(0, 1)
                emit_a(0, 2)
                emit_a(0, 3)
                emit_t0(1, 0)
                emit_t0(1, 1)
                for s in range(len(units)):
                    H, t = units[s]
                    if t == 0 and H >= 1 and H + 2 < N_HALVES:
                        emit_in(H + 2)
                    emit_t1(H, t)
                    # inject T0 tile (h', j') solving (h'-1)*4 + j' - 2 == s
                    j2 = (s + 2) % 4
                    h2 = (s + 2 - j2) // 4 + 1
                    if h2 < N_HALVES:
                        emit_t0(h2, j2)
                    if s >= 1:
                        emit_b(*units[s - 1])
                    if s + 2 < len(units):
                        H3, t3 = units[s + 2]
                        emit_a(H3, 2 * t3)
                        emit_a(H3, 2 * t3 + 1)
                emit_b(*units[-1])

            import contextlib

            loop_ctx = (
                tc.For_i(0, repeat) if hw_loop and repeat > 1
                else contextlib.nullcontext()
            )
            with loop_ctx:
                for _ in range(1 if hw_loop else repeat):
                    emit_pass()

    nc.compile()
    return nc


_CACHE = {}


def _get_program():
    if "nc" not in _CACHE:
        _CACHE["nc"] = build_program()
    return _CACHE["nc"]


def make_in_maps(x: np.ndarray) -> list:
    import ml_dtypes

    x_flat = np.ascontiguousarray(
        x.reshape(16384, DIM).astype(ml_dtypes.bfloat16)
    )
    h128, k32, ident = _constants()
    return [
        {
            "x": x_flat[i * ROWS_PER_CORE : (i + 1) * ROWS_PER_CORE],
            "h128": h128,
            "k32": k32,
            "ident": ident,
        }
        for i in range(N_CORES)
    ]


def assemble_output(res: dict) -> np.ndarray:
    """Undo the packed per-core output layout: y_packed[core] is
    [(c'*4+rr'), half, g*128+k'] bf16; global row = half*128 + g*4 + rr'."""
    yp = np.asarray(res["y"]).reshape(N_CORES, 32, 4, N_HALVES, 32, 128)
    y = yp.transpose(0, 3, 4, 2, 1, 5).reshape(16384, 4096)
    return y.astype(np.float32).reshape(4, 4096, 4096)


def kernel(x: np.ndarray, _trace: bool = False, _trace_kwargs=None) -> np.ndarray:
    assert x.shape == (4, 4096, 4096), x.shape
    in_maps = make_in_maps(x)

    nc = _get_program()
    res = bass_utils.run_bass_kernel_spmd(
        nc,
        in_maps,
        core_ids=list(range(N_CORES)),
        trace=_trace,
        **(_trace_kwargs or {}),
    )
    y = assemble_output({"y": np.stack([res.results[i]["y"] for i in range(N_CORES)])})
    if _trace:
        _CACHE["last_result"] = res
    return y


# revision 42
# speedup vs baseline: 1.5622x; 1.1167x over previous
"""FWHT (normalized Walsh-Hadamard transform) over the last dim of x[4,4096,4096].

Rows are independent -> shard 16384 rows across 8 NeuronCores (2048 each).
Per row, H_4096 = H_32 (x) H_128 (Sylvester Kronecker factorization); with
the row viewed as X[c, k] (c in [0,32), k in [0,128)):

    y[c'*128+k'] = (1/64) * sum_{c,k} H32[c',c] H128[k',k] X[c,k]

All storage and PE streams are bf16 (inputs cast on host; rel err ~2e-3,
inside the 2e-2 gate); PSUM accumulation is fp32.  Work is organized in
16 half-blocks of 128 rows, software-pipelined so that every engine
(PE / DVE / ACT / DMA) stays busy:

  T0 :  PE transposes x[r, (c,k)] tiles  -> Z[k | (g,c,rr)]   bf16  (DVE drain,
        scattered so stage A output is g-contiguous; r = g*4+rr)
  A  :  matmul H128/8 x Z                -> B[k' | (g,c,rr)]  (ACT drain, cast)
  T1 :  PE transposes B[:, g*128:+128]   -> Z2[(c,rr) | k']   bf16  (DVE drain)
  B  :  matmul kron(H32,I4)/8 x Z2       -> OUT[(c',rr') | (g,k')] f32
  out:  4 DMAs per half (one per rr), 512B-contiguous k' runs.

Emission interleaves, per half-block: [A,A,T1+B] x4 with the NEXT half's
T0 tiles injected each iteration, so DVE (bf16 drains) and ACT (fp32
drains) overlap instead of alternating in phases.
"""

import numpy as np

try:
    import concourse.bass as bass  # noqa: F401
except ImportError:
    import sys

    sys.path.insert(0, "/opt/trn_rl_repo")

from concourse import bacc, bass, bass_utils, tile
from concourse import mybir

F32 = mybir.dt.float32
BF16 = mybir.dt.bfloat16

N_CORES = 8
ROWS_PER_CORE = 2048
DIM = 4096
N_BLOCKS = 8
BLOCK_ROWS = 256
N_HALVES = 16  # 128-row halves per pass


def _hadamard(n: int) -> np.ndarray:
    h = np.array([[1.0]], dtype=np.float64)
    while h.shape[0] < n:
        h = np.block([[h, h], [h, -h]])
    return h


def _constants():
    import ml_dtypes

    bf = ml_dtypes.bfloat16
    h128 = (_hadamard(128) * 0.125).astype(bf)
    # stage-B stationary: contraction index p=(c*4+rr), output q=(c'*4+rr')
    k32 = (np.kron(_hadamard(32), np.eye(4)) * 0.125).astype(bf)
    ident = np.eye(128).astype(bf)
    return h128, k32, ident


def build_program(repeat: int = 1, hw_loop: bool = False):
    nc = bacc.Bacc(
        "TRN2",
        target_bir_lowering=False,
        debug=False,
        enable_asserts=False,
    )

    x_d = nc.dram_tensor("x", [ROWS_PER_CORE, DIM], BF16, kind="ExternalInput").ap()
    h128_d = nc.dram_tensor("h128", [128, 128], BF16, kind="ExternalInput").ap()
    k32_d = nc.dram_tensor("k32", [128, 128], BF16, kind="ExternalInput").ap()
    id_d = nc.dram_tensor("ident", [128, 128], BF16, kind="ExternalInput").ap()
    # y is written bf16 in the SBUF-native layout [(c'*4+rr'), half, g*128+k']
    # (fully contiguous per partition -> 8KB DMA descriptors); the host
    # permutes/upcasts during unshard.
    y_d = nc.dram_tensor(
        "y", [128, N_HALVES, DIM], BF16, kind="ExternalOutput"
    ).ap()

    with tile.TileContext(nc) as tc:
        with (
            tc.tile_pool(name="const", bufs=1) as const_pool,
            tc.tile_pool(name="xin", bufs=8) as x_pool,
            tc.tile_pool(name="zbuf", bufs=3) as z_pool,
            tc.tile_pool(name="bbuf", bufs=2) as b_pool,
            tc.tile_pool(name="z2buf", bufs=4) as z2_pool,
            tc.tile_pool(name="obuf", bufs=4) as o_pool,
            tc.tile_pool(name="ps_t", bufs=3, space="PSUM") as t_psum,
            tc.tile_pool(name="ps_a", bufs=3, space="PSUM") as a_psum,
            tc.tile_pool(name="ps_b", bufs=2, space="PSUM") as b_psum,
        ):
            h128_t = const_pool.tile([128, 128], BF16)
            k32_t = const_pool.tile([128, 128], BF16)
            id_t = const_pool.tile([128, 128], BF16)
            nc.sync.dma_start(h128_t[:], h128_d)
            nc.sync.dma_start(k32_t[:], k32_d)
            nc.sync.dma_start(id_t[:], id_d)

            xts = {}  # half -> x tile [128, 4096]
            z_tiles = {}  # half -> Z tile
            bb_tiles = {}
            out_tiles = {}

            def emit_in(h):
                # split across both HWDGE queues to halve arrival latency
                b, rs = divmod(h, 2)
                t = x_pool.tile([128, DIM], BF16, name=f"x_{h}", tag="x")
                r0 = (b % N_BLOCKS) * BLOCK_ROWS + rs * 128
                half = DIM // 2
                nc.sync.dma_start(t[:, 0:half], x_d[r0 : r0 + 128, 0:half])
                nc.scalar.dma_start(t[:, half:], x_d[r0 : r0 + 128, half:])
                xts[h] = t

            def emit_t0(h, j):
                # j-th T0 psum tile = 8 transposes, c in [8j, 8j+8)
                if j == 0:
                    z_tiles[h] = z_pool.tile([128, DIM], BF16, name=f"z_{h}", tag="z")
                z = z_tiles[h]
                xt = xts.pop(h) if j == 3 else xts[h]
                ps = t_psum.tile([128, 1024], BF16, name=f"t0ps_{h}_{j}", tag="tps")
                for i in range(8):
                    c = j * 8 + i
                    nc.tensor.transpose(
                        ps[:, i * 128 : (i + 1) * 128],
                        xt[:, c * 128 : (c + 1) * 128],
                        id_t[:],
                    )
                zr = z[:].rearrange("p (g c rr) -> p c g rr", g=32, c=32, rr=4)
                psr = ps[:].rearrange("p (c g rr) -> p c g rr", c=8, g=32, rr=4)
                nc.vector.tensor_copy(zr[:, j * 8 : (j + 1) * 8], psr)

            def emit_a(h, m):
                if m == 0:
                    bb_tiles[h] = b_pool.tile([128, DIM], BF16, name=f"bb_{h}", tag="bb")
                z = z_tiles[h]
                bb = bb_tiles[h]
                ps = a_psum.tile([128, 512], F32, name=f"aps_{h}_{m}", tag="aps")
                nc.tensor.matmul(ps[:], h128_t[:], z[:, m * 512 : (m + 1) * 512])
                nc.scalar.copy(bb[:, m * 512 : (m + 1) * 512], ps[:])
                if m == 7:
                    del z_tiles[h]

            z2_tiles = {}

            def emit_t1(h, t):
                bb = bb_tiles[h]
                ps = t_psum.tile([128, 1024], BF16, name=f"t1ps_{h}_{t}", tag="tps")
                for j in range(8):
                    g = t * 8 + j
                    nc.tensor.transpose(
                        ps[:, j * 128 : (j + 1) * 128],
                        bb[:, g * 128 : (g + 1) * 128],
                        id_t[:],
                    )
                z2 = z2_pool.tile([128, 1024], BF16, name=f"z2_{h}_{t}", tag="z2")
                nc.vector.tensor_copy(z2[:], ps[:])
                z2_tiles[(h, t)] = z2
                if t == 3:
                    del bb_tiles[h]

            def emit_b(h, t):
                if t == 0:
                    out_tiles[h] = o_pool.tile([128, DIM], BF16, name=f"out_{h}", tag="out")
                out = out_tiles[h]
                z2 = z2_tiles.pop((h, t))
                for hh in range(2):
                    psb = b_psum.tile([128, 512], F32, name=f"bps_{h}_{t}_{hh}", tag="bps")
                    nc.tensor.matmul(
                        psb[:], k32_t[:], z2[:, hh * 512 : (hh + 1) * 512]
                    )
                    dst = out[:, t * 1024 + hh * 512 : t * 1024 + (hh + 1) * 512]
                    # ~4/9 of fp32 B-drains to DVE balances DVE/ACT busy time
                    if ((h * 4 + t) * 2 + hh) % 9 < 4:
                        nc.vector.tensor_copy(dst, psb[:])
                    else:
                        nc.scalar.copy(dst, psb[:])
                if t == 3:
                    emit_out(h)

            def emit_out(h):
                b, rs = divmod(h, 2)
                out = out_tiles.pop(h)
                eng = nc.sync if h % 2 == 0 else nc.scalar
                eng.dma_start(y_d[:, h % N_HALVES, :], out[:])

            def emit_pass():
                # 64 pipeline slots; unit s = (H=s//4, t=s%4).  Leads/lags:
                # T0 of half h is injected 2 slots ahead of h's first A-pair;
                # A-pair for unit u is emitted 2 slots before its T1;
                # B runs 1 slot behind its T1.
                units = [(H, t) for H in range(N_HALVES) for t in range(4)]

                emit_in(0)
                emit_in(1)
                emit_in(2)
                for j in range(4):
                    emit_t0(0, j)
                emit_a(0, 0)
                emit_a(0, 1)
                emit_a(0, 2)
                emit_a(0, 3)
                emit_t0(1, 0)
                emit_t0(1, 1)
                for s in range(len(units)):
                    H, t = units[s]
                    if t == 0 and H >= 1 and H + 2 < N_HALVES:
                        emit_in(H + 2)
                    emit_t1(H, t)
                    # inject T0 tile (h', j') solving (h'-1)*4 + j' - 2 == s
                    j2 = (s + 2) % 4
                    h2 = (s + 2 - j2) // 4 + 1
                    if h2 < N_HALVES:
                        emit_t0(h2, j2)
                    if s >= 1:
                        emit_b(*units[s - 1])
                    if s + 2 < len(units):
                        H3, t3 = units[s + 2]
                        emit_a(H3, 2 * t3)
                        emit_a(H3, 2 * t3 + 1)
                emit_b(*units[-1])

            import contextlib

            loop_ctx = (
                tc.For_i(0, repeat) if hw_loop and repeat > 1
                else contextlib.nullcontext()
            )
            with loop_ctx:
                for _ in range(1 if hw_loop else repeat):
                    emit_pass()

    nc.compile()
    return nc


_CACHE = {}


def _get_program():
    if "nc" not in _CACHE:
        _CACHE["nc"] = build_program()
    return _CACHE["nc"]


def make_in_maps(x: np.ndarray) -> list:
    import ml_dtypes

    x_flat = np.ascontiguousarray(
        x.reshape(16384, DIM).astype(ml_dtypes.bfloat16)
    )
    h128, k32, ident = _constants()
    return [
        {
            "x": x_flat[i * ROWS_PER_CORE : (i + 1) * ROWS_PER_CORE],
            "h128": h128,
            "k32": k32,
            "ident": ident,
        }
        for i in range(N_CORES)
    ]


def assemble_output(res: dict) -> np.ndarray:
    """Undo the packed per-core output layout: y_packed[core] is
    [(c'*4+rr'), half, g*128+k'] bf16; global row = half*128 + g*4 + rr'."""
    yp = np.asarray(res["y"]).reshape(N_CORES, 32, 4, N_HALVES, 32, 128)
    y = yp.transpose(0, 3, 4, 2, 1, 5).reshape(16384, 4096)
    return y.astype(np.float32).reshape(4, 4096, 4096)


def kernel(x: np.ndarray, _trace: bool = False, _trace_kwargs=None) -> np.ndarray:
    assert x.shape == (4, 4096, 4096), x.shape
    in_maps = make_in_maps(x)

    nc = _get_program()
    res = bass_utils.run_bass_kernel_spmd(
        nc,
        in_maps,
        core_ids=list(range(N_CORES)),
        trace=_trace,
        **(_trace_kwargs or {}),
    )
    y = assemble_output({"y": np.stack([res.results[i]["y"] for i in range(N_CORES)])})
    if _trace:
        _CACHE["last_result"] = res
    return y
